# revision 4
# baseline (speedup 1.0000x reference)
"""LoRA linear layer (base GEMM + low-rank path) on 8 Trainium2 NeuronCores.

Computes  Y = X @ W^T + ((X*mask) @ A) @ B  (SCALE = 32/32 = 1.0) for
X [4, 2048, 4096], W [4096, 4096], A [4096, 32], B [32, 4096].

Sharding: data-parallel over tokens. X/mask flattened to [8192, 4096] and
split into 8 shards of 1024 tokens; W/A/B replicated per core.

Per-core kernel (Tile framework):
  Phase 0: stream x/mask tiles, PE-transpose x into a resident x^T SBUF
    store [128, 32ic, 1024m]; multiply mask in, transpose, and contract with
    A on the fly to produce lora1^T = A^T @ (x*m)^T  [32, 1024] in SBUF.
  Main loop over 8 output chunks of 512 features:
    - transpose phase: stream W[oc] natural tiles, PE-transpose into a
      w^T cache [128, 32ic, 512] in SBUF.
    - matmul phase: per 128-token tile, one K=32 matmul folds the lora path
      (lhsT=lora1^T slice, rhs=B[:, oc]) into PSUM, then 32 K=128 matmuls
      accumulate x^T.T @ w^T; copy PSUM -> SBUF -> DRAM.

Matmul/data dtype is fp32 by default; set MM_DT to float32r for 4x matmul
throughput at ~1.5e-4 relative error.
"""

import os

import numpy as np

import concourse.bass as bass
import concourse.mybir as mybir
import concourse.tile as tile
from concourse.masks import make_identity
from concourse.vector_clock import ScopedClock

# ---------------------------------------------------------------- constants
N_CORES = 8
B_, S, D = 4, 2048, 4096
M = B_ * S          # 8192 tokens total
MS = M // N_CORES   # 1024 tokens per core
R = 32              # lora rank
P = 128
IC = D // P         # 32 contraction chunks
MT = MS // P        # 8 token tiles per core
ONX = 512           # output-feature chunk (one PSUM bank of fp32)
OC = D // ONX       # 8 output chunks

FP32 = mybir.dt.float32
FP32R = mybir.dt.float32r
MM_DT = FP32R if os.environ.get("LORA_MM_DT", "fp32r") == "fp32r" else FP32


# ------------------------------------------------- walrus sync-wait compat
def _split_multi_waits(nc, max_waits: int = 1):
    """neuronxcc's walrus codegen accepts at most one semaphore wait per
    instruction; Tile's internal lowering assumes multi-waits get split
    later.  Split them here: extra waits move onto wait-only EventSemaphore
    instructions inserted right before the instruction on the same engine."""
    for f in nc.m.functions:
        for bb in f.blocks:
            il = bb.instructions
            k = 0
            while k < len(il):
                inst = il[k]
                si = inst.sync_info
                if si is not None and len(si.on_wait) > max_waits:
                    waits = list(si.on_wait)
                    si.on_wait = waits[:max_waits]
                    extra = waits[max_waits:]
                    pos = 0
                    for j in range(0, len(extra), max_waits):
                        evs = mybir.InstEventSemaphore(
                            name=f"{inst.name}-wsplit{j}",
                            engine=inst.engine,
                            ins=[],
                            outs=[],
                            sync_info=mybir.SyncInfo(
                                on_wait=extra[j : j + max_waits], on_update=[]
                            ),
                        )
                        il.insert(k + pos, evs)
                        pos += 1
                    k += pos
                k += 1


class _WalrusTileContext(tile.TileContext):
    def _drain_and_barrier(self, tick_clock, wait_clock):
        nc = self.nc
        drain_inst = nc.sync.drain()
        wait_clock.add_sem_waits(
            drain_inst.ins, ScopedClock({None: tick_clock.global_clock})
        )
        nc.all_engine_barrier()
        assert self.sems is not None
        popped = nc._tile_sem_poison_stack.pop()
        assert popped is self._sem_poison
        nc.clear_and_free_semaphores(list(self.sems.allocated().values()))
        nc.all_engine_barrier()

    def __exit__(self, exc_type, exc_value, traceback):
        ret = super().__exit__(exc_type, exc_value, traceback)
        if exc_type is None:
            _split_multi_waits(self.nc)
        return ret


# ----------------------------------------------------------- kernel build
def _build_nc():
    nc = bass.Bass(dynamic_dma_scratch_size=512)
    xs = nc.dram_tensor("xs", [MS, D], FP32, kind="ExternalInput")
    ms = nc.dram_tensor("ms", [MS, D], FP32, kind="ExternalInput")
    W = nc.dram_tensor("W", [D, D], FP32, kind="ExternalInput")
    A = nc.dram_tensor("A", [D, R], FP32, kind="ExternalInput")
    Bm = nc.dram_tensor("Bm", [R, D], FP32, kind="ExternalInput")
    ys = nc.dram_tensor("ys", [MS, D], FP32, kind="ExternalOutput")

    with _WalrusTileContext(nc) as tc:
        with tc.tile_pool(name="res", bufs=1) as res:
            # resident tensors
            xT = res.tile([P, IC, MS], MM_DT)     # x^T store: [i, ic, m]
            lora1T = res.tile([R, MS], MM_DT)     # (xm @ A)^T: [r, m]
            ident = res.tile([P, P], FP32)
            make_identity(nc, ident)

            # ---------------- phase 0: build xT and lora1T ----------------
            with (
                tc.tile_pool(name="p0", bufs=2) as p0,
                tc.tile_pool(name="p0psum", bufs=4, space="PSUM") as p0psum,
                tc.tile_pool(name="p0lora", bufs=2, space="PSUM") as p0lora,
            ):
                if MM_DT == FP32:
                    a_sb = p0.tile([P, IC, R], FP32, tag="asb")
                    nc.sync.dma_start(
                        a_sb[:], A[:, :].rearrange("(ic p) r -> p ic r", p=P)
                    )
                else:
                    a_st = p0.tile([P, IC, R], FP32, tag="ast")
                    nc.sync.dma_start(
                        a_st[:], A[:, :].rearrange("(ic p) r -> p ic r", p=P)
                    )
                    a_sb = p0.tile([P, IC, R], MM_DT, tag="asb")
                    nc.vector.tensor_copy(a_sb[:], a_st[:])

                for mg in range(2):  # token half-shards of 512
                    lora_ps = p0lora.tile([R, 512], FP32, tag="lorap")
                    for iq in range(8):  # feature chunks of 512
                        x_nat = p0.tile([P, 4, 512], FP32, tag="xnat")
                        m_nat = p0.tile([P, 4, 512], FP32, tag="mnat")
                        xsl = slice(mg * 512, (mg + 1) * 512)
                        isl = slice(iq * 512, (iq + 1) * 512)
                        nc.sync.dma_start(
                            x_nat[:], xs[xsl, isl].rearrange("(s p) i -> p s i", p=P)
                        )
                        nc.sync.dma_start(
                            m_nat[:], ms[xsl, isl].rearrange("(s p) i -> p s i", p=P)
                        )
                        for ic4 in range(4):
                            k = iq * 4 + ic4
                            cs = slice(ic4 * P, (ic4 + 1) * P)
                            tp = p0psum.tile([P, 512], FP32, tag="tp")
                            for s in range(4):
                                nc.tensor.transpose(
                                    tp[:, s * P : (s + 1) * P], x_nat[:, s, cs], ident
                                )
                            nc.vector.tensor_copy(
                                xT[:, k, mg * 512 : (mg + 1) * 512], tp[:]
                            )
                        # xm = x * mask (in place over the mask tile)
                        nc.vector.tensor_mul(m_nat[:], x_nat[:], m_nat[:])
                        for ic4 in range(4):
                            k = iq * 4 + ic4
                            cs = slice(ic4 * P, (ic4 + 1) * P)
                            tpm = p0psum.tile([P, 512], FP32, tag="tp")
                            for s in range(4):
                                nc.tensor.transpose(
                                    tpm[:, s * P : (s + 1) * P], m_nat[:, s, cs], ident
                                )
                            xmt = p0.tile([P, 512], MM_DT, tag="xmt")
                            nc.vector.tensor_copy(xmt[:], tpm[:])
                            nc.tensor.matmul(
                                lora_ps[:],
                                a_sb[:, k, :],
                                xmt[:],
                                start=(k == 0),
                                stop=(k == IC - 1),
                            )
                    nc.vector.tensor_copy(
                        lora1T[:, mg * 512 : (mg + 1) * 512], lora_ps[:]
                    )

            # ---------------- main loop over output chunks ----------------
            main_reps = int(os.environ.get("LORA_MAIN_REPS", "1"))
            with (
                tc.tile_pool(name="wt", bufs=1) as wt_pool,
                tc.tile_pool(name="mstage", bufs=2) as mstage,
                tc.tile_pool(name="mpsum", bufs=8, space="PSUM") as mpsum,
            ):
                for oc in [c for c in range(OC) for _ in range(main_reps)]:
                    osl = slice(oc * ONX, (oc + 1) * ONX)
                    if MM_DT == FP32:
                        b_sb = mstage.tile([R, ONX], FP32, tag="bsb")
                        nc.sync.dma_start(b_sb[:], Bm[:, osl])
                    else:
                        b_st = mstage.tile([R, ONX], FP32, tag="bst")
                        nc.sync.dma_start(b_st[:], Bm[:, osl])
                        b_sb = mstage.tile([R, ONX], MM_DT, tag="bsb")
                        nc.vector.tensor_copy(b_sb[:], b_st[:])

                    wt = wt_pool.tile([P, IC, ONX], MM_DT, tag="wt")
                    for ic in range(IC):
                        wn = mstage.tile([P, 4, P], FP32, tag="wn")
                        nc.sync.dma_start(
                            wn[:],
                            W[osl, ic * P : (ic + 1) * P].rearrange(
                                "(s p) i -> p s i", p=P
                            ),
                        )
                        tp = mpsum.tile([P, ONX], FP32, tag="bank")
                        for s in range(4):
                            nc.tensor.transpose(
                                tp[:, s * P : (s + 1) * P], wn[:, s, :], ident
                            )
                        nc.vector.tensor_copy(wt[:, ic, :], tp[:])

                    for mt in range(MT):
                        msl = slice(mt * P, (mt + 1) * P)
                        ps = mpsum.tile([P, ONX], FP32, tag="bank")
                        nc.tensor.matmul(
                            ps[:], lora1T[:, msl], b_sb[:], start=True, stop=False
                        )
                        for ic in range(IC):
                            nc.tensor.matmul(
                                ps[:],
                                xT[:, ic, msl],
                                wt[:, ic, :],
                                start=False,
                                stop=(ic == IC - 1),
                            )
                        st = mstage.tile([P, ONX], FP32, tag="st")
                        nc.vector.tensor_copy(st[:], ps[:])
                        nc.sync.dma_start(ys[msl, osl], st[:])

    return nc


# ------------------------------------------------------ cached executor
_EXEC = None


def _get_exec():
    """Compile once; return (fn, n_params, in_names, out_names, out_shapes).

    fn takes concatenated global inputs (n_cores*dim0, ...) plus donated
    zero output buffers, returns concatenated outputs. Mirrors
    bass2jax.run_bass_via_pjrt's multi-core path but caches the jit."""
    global _EXEC
    if _EXEC is not None:
        return _EXEC

    import jax
    from concourse import bass2jax
    from jax.experimental.shard_map import shard_map
    from jax.sharding import Mesh, PartitionSpec

    nc = _build_nc()
    bass2jax.install_neuronx_cc_hook()
    partition_name = nc.partition_id_tensor.name if nc.partition_id_tensor else None

    in_names, out_names, out_avals, zero_shapes = [], [], [], []
    for alloc in nc.m.functions[0].allocations:
        if not isinstance(alloc, mybir.MemoryLocationSet):
            continue
        name = alloc.memorylocations[0].name
        if alloc.kind == "ExternalInput":
            if name != partition_name:
                in_names.append(name)
        elif alloc.kind == "ExternalOutput":
            shape = tuple(alloc.tensor_shape)
            dtype = mybir.dt.np(alloc.dtype)
            out_names.append(name)
            out_avals.append(jax.core.ShapedArray(shape, dtype))
            zero_shapes.append((shape, dtype))
    n_params = len(in_names)
    all_in_names = in_names + out_names
    if partition_name is not None:
        all_in_names.append(partition_name)
    donate = tuple(range(n_params, n_params + len(out_names)))

    def _body(*args):
        operands = list(args)
        if partition_name is not None:
            operands.append(bass2jax.partition_id_tensor())
        outs = bass2jax._bass_exec_p.bind(
            *operands,
            out_avals=tuple(out_avals),
            in_names=tuple(all_in_names),
            out_names=tuple(out_names),
            lowering_input_output_aliases=(),
            sim_require_finite=True,
            sim_require_nnan=True,
            nc=nc,
        )
        return tuple(outs)

    devices = jax.devices()[:N_CORES]
    mesh = Mesh(np.asarray(devices), ("core",))
    specs = (PartitionSpec("core"),) * (n_params + len(out_names))
    fn = jax.jit(
        shard_map(
            _body,
            mesh=mesh,
            in_specs=specs,
            out_specs=(PartitionSpec("core"),) * len(out_names),
            check_rep=False,
        ),
        donate_argnums=donate,
        keep_unused=True,
    )
    _EXEC = (fn, n_params, in_names, out_names, zero_shapes)
    return _EXEC


def _shard_inputs(x, W, A, B, drop_mask):
    """Full inputs -> dict of concatenated per-core arrays (axis 0)."""
    xf = np.ascontiguousarray(x, dtype=np.float32).reshape(M, D)
    mf = np.ascontiguousarray(drop_mask, dtype=np.float32).reshape(M, D)
    W = np.ascontiguousarray(W, dtype=np.float32)
    A = np.ascontiguousarray(A, dtype=np.float32)
    B = np.ascontiguousarray(B, dtype=np.float32)
    return {
        "xs": xf,                                  # already (8*1024, D)
        "ms": mf,
        "W": np.concatenate([W] * N_CORES, axis=0),
        "A": np.concatenate([A] * N_CORES, axis=0),
        "Bm": np.concatenate([B] * N_CORES, axis=0),
    }


def _run(concat_inputs):
    import jax.numpy as jnp

    fn, n_params, in_names, out_names, zero_shapes = _get_exec()
    args = [concat_inputs[name] for name in in_names]
    zeros = [
        jnp.zeros((N_CORES * s[0], *s[1:]), dt) for (s, dt) in zero_shapes
    ]
    outs = fn(*args, *zeros)
    return {name: np.asarray(o) for name, o in zip(out_names, outs)}


def kernel(x, W, A, B, drop_mask):
    out = _run(_shard_inputs(x, W, A, B, drop_mask))
    return out["ys"].reshape(B_, S, D)


# -------------------------------------------------- timing hook for tests
def timed_run(x, W, A, B, drop_mask, iters=5):
    """Returns (result, best_wall_ns) over `iters` steady-state executions
    with device-resident inputs."""
    import time

    import jax
    import jax.numpy as jnp

    fn, n_params, in_names, out_names, zero_shapes = _get_exec()
    concat = _shard_inputs(x, W, A, B, drop_mask)
    args = [jax.device_put(concat[name]) for name in in_names]
    for a in args:
        a.block_until_ready()

    def one_call():
        zeros = [
            jnp.zeros((N_CORES * s[0], *s[1:]), dt) for (s, dt) in zero_shapes
        ]
        for z in zeros:
            z.block_until_ready()
        t0 = time.perf_counter()
        outs = fn(*args, *zeros)
        for o in outs:
            o.block_until_ready()
        return time.perf_counter() - t0, outs

    one_call()  # warm-up / compile
    best, outs = None, None
    for _ in range(iters):
        dt, o = one_call()
        if best is None or dt < best:
            best, outs = dt, o
    res = {name: np.asarray(o) for name, o in zip(out_names, outs)}
    return res["ys"].reshape(B_, S, D), int(best * 1e9)
